# revision 33
# baseline (speedup 1.0000x reference)
"""Trainium2 Bass kernel for AttentionDenseBlock (SE gate + offset conv + deform conv + tanh).

Strategy (per core, data-parallel over batch: 1 sample/core on 8 cores):
  - Weight transposes (w_conv -> [c,k,o], w_off de-interleave, fc1/fc2) are
    done host-side in numpy; the device only casts f32 -> bf16.
  - The SE channel gate s is folded into the conv weights (wT, woffT scaled
    per-channel on the contraction side), so the x pipeline (cast + shifted
    copies) never waits on the SE chain.
  - x is stored as five column-pre-shifted, zero-filled 56-wide bf16 tiles
    (dw in -2..2): every window read is a fully contiguous [128, 2, 784]
    slice, which keeps the DVE modulate multiplies near their 2x-mode
    roofline (no per-row segment overhead).
  - Offset conv: 9 shifted bf16 matmuls per chunk accumulating in PSUM;
    bilinear tap-weight maps go to per-quarter DRAM tiles and are read back
    with partition-broadcast DMAs (3 tap rows per trigger).
  - Deform conv: |offset|<1 decomposes exactly into a static 3x3-tap stencil
    per kernel position with data-dependent weights.  6 positions run
    "direct" (one matmul event per tap, PSUM accumulates); 3 positions are
    z-accumulated into one [128, 3, 2, 784] tile via gpsimd-issued
    accumulate-DMAs (one per tap row).  The modulates all run on DVE
    (concurrent gpsimd elementwise work contends with DVE for SBUF);
    z-chains and next-quarter map products are interleaved tap-wise into
    the direct stream one quarter ahead so no engine blocks the PE.
  - Epilogue: tanh(psum + b_conv) fused on ACT -> DMA out.
"""

import os
import sys
from contextlib import ExitStack

import numpy as np

sys.path.insert(0, "/opt/trn_rl_repo")

import concourse.bass as bass
import concourse.bacc as bacc
import concourse.mybir as mybir
import concourse.tile as tile

B, C, O, H, W = 8, 256, 256, 56, 56
KH = KW = 3
K2 = 9
HP = H + 4            # rows zero-padded by 2 for the 5x5 shift range
HW = H * W
QROWS = 14            # rows per quarter
QN = HW // 4          # 784 spatial positions per quarter
NN = QN // 2          # 392 = offset-conv matmul N-chunk
RED = 16              # SE reduction dim

Z_KS = (0, 8)         # kernel positions handled via z-accumulation
NZ = len(Z_KS)
Z_IDX = {0: 0, 8: 1}
# PE event order within a quarter: direct positions carry PSUM start/stop.
# Quarter 0 puts z events last (its z-chain runs during the offset conv);
# later quarters get z mid-stream (chain ran a quarter ahead).
EV_STD = [("d", 1), ("d", 2), ("d", 3), ("z", 0), ("d", 4), ("d", 5),
          ("z", 8), ("d", 6), ("d", 7)]
EV_Q0 = [("d", 1), ("d", 2), ("d", 3), ("d", 4), ("d", 5), ("d", 6),
         ("z", 0), ("z", 8), ("d", 7)]
TAPS = [(r, s) for r in range(3) for s in range(3)]
DWS = (-2, -1, 0, 1, 2)

F32 = mybir.dt.float32
BF16 = mybir.dt.bfloat16
AF = mybir.ActivationFunctionType
ALU = mybir.AluOpType

LAST_RESULT = None


def _bcast_ap(base, extra_dims):
    """AP reading `base` with extra broadcast/reshape free dims."""
    return bass.AP(tensor=base.tensor, offset=base.offset,
                   ap=[list(base.ap[0])] + [list(d) for d in extra_dims])


def build():
    nc = bacc.Bacc()
    x_d = nc.dram_tensor("x", (1, C, H, W), F32, kind="ExternalInput")
    # host-pre-transposed weights
    wTt_d = nc.dram_tensor("wTt", (2, 128, K2, O), F32, kind="ExternalInput")
    woffTt_d = nc.dram_tensor("woffTt", (2, 128, K2, 64), F32, kind="ExternalInput")
    boffd_d = nc.dram_tensor("boffd", (64,), F32, kind="ExternalInput")
    bconv_d = nc.dram_tensor("b_conv", (O,), F32, kind="ExternalInput")
    fc1t_d = nc.dram_tensor("fc1t", (2, 128, RED), F32, kind="ExternalInput")
    fc2t_d = nc.dram_tensor("fc2t", (RED, C), F32, kind="ExternalInput")
    out_d = nc.dram_tensor("out", (1, O, H, W), F32, kind="ExternalOutput")

    # round-robin DMA-trigger queues for broadcast reads (HWDGE only; the
    # gpsimd queue is reserved for the z-accumulate DMAs)
    def _rr_engines():
        while True:
            yield nc.sync
            yield nc.scalar
    rr = _rr_engines()

    with tile.TileContext(nc) as tc, ExitStack() as ctx:
        singles = ctx.enter_context(tc.tile_pool(name="singles", bufs=1))
        dpool = ctx.enter_context(tc.tile_pool(name="dpool", bufs=1, space="DRAM"))
        mapc = ctx.enter_context(tc.tile_pool(name="mapc", bufs=3))
        mtpool = ctx.enter_context(tc.tile_pool(name="mtpool", bufs=1))

        # ---- persistent tiles ----
        # column-pre-shifted x copies in ONE tile: xsh_all[p, dw+2, cc, h, w]
        # = x at column w+dw, 0 outside (unscaled; SE gate folded into
        # weights).  One contiguous tile lets a single DVE op cover 3 taps
        # (regular plane/row stride across the group).
        xsh_all = singles.tile([128, 5, 2, HP, W], BF16)
        xsh = {dw: xsh_all[:, dw + 2, :, :, :] for dw in DWS}
        wT = singles.tile([128, 2, K2, O], BF16)         # [c, cc, k, o] (SE-scaled)
        woffT = singles.tile([128, 2, K2, 64], BF16)     # dy w in [0:9], dx in [32:41]
        fc1T = singles.tile([128, 2, RED], F32)
        fc2T = singles.tile([128, C], F32)
        bconv = singles.tile([128, 2], F32)
        boff = singles.tile([64, 1], F32)
        boffn = singles.tile([64, 1], F32)
        y_se = singles.tile([128, 2, 1], F32)
        h_se = singles.tile([128, 1], F32)
        s_se = singles.tile([128, 2, 1], F32)
        maps_dram = [dpool.tile([K2 * K2, QN], BF16, name=f"maps{q}")
                     for q in range(4)]

        # per-chunk bilinear tap-weight rows (alive into the main loop)
        def chunk_tiles():
            return {nm: mapc.tile([K2, NN], BF16, tag=nm, name=nm)
                    for nm in ("wy0", "wy2", "wx0", "wx2")}
        wyx = {}   # (q, nn) -> dict of tiles

        def emit_map_chunk(q, nn):
            """9 tap-map products for one chunk -> one batched DRAM write."""
            ct = wyx[(q, nn)]
            # wy1 = 1 - (wy0 + wy2); wx1 likewise (scratch tiles)
            wy1 = mtpool.tile([K2, NN], BF16, tag="wy1", name="wy1")
            wx1 = mtpool.tile([K2, NN], BF16, tag="wx1", name="wx1")
            nc.vector.tensor_add(wy1[:, :], ct["wy0"][:, :], ct["wy2"][:, :])
            nc.scalar.activation(wy1[:, :], wy1[:, :], AF.Copy, scale=-1.0, bias=1.0)
            nc.vector.tensor_add(wx1[:, :], ct["wx0"][:, :], ct["wx2"][:, :])
            nc.scalar.activation(wx1[:, :], wx1[:, :], AF.Copy, scale=-1.0, bias=1.0)
            wys = (ct["wy0"], wy1, ct["wy2"])
            wxs = (ct["wx0"], wx1, ct["wx2"])
            mtmp = mtpool.tile([K2, K2, NN], BF16)
            for r in range(3):
                for s in range(3):
                    nc.vector.tensor_mul(mtmp[:, 3 * r + s, :],
                                         wys[r][:, :], wxs[s][:, :])
            md = maps_dram[q][0:1, 0:1]
            next(rr).dma_start(
                out=bass.AP(tensor=md.tensor, offset=md.offset + nn * NN,
                            ap=[[QN, K2], [K2 * QN, K2], [1, NN]]),
                in_=mtmp[:, :, :])

        with tc.tile_pool(name="ph1", bufs=1) as ph1, \
             tc.tile_pool(name="psum_pre", bufs=2, space="PSUM") as psum_pre:

            xs_pad = ph1.tile([128, 2, HP, W + 4], F32)
            wTf = ph1.tile([128, 2, K2, O], F32)
            woffTf = ph1.tile([128, 2, K2, 64], F32)

            # zero only the border rows and shifted-in edge columns of the
            # xsh planes (interiors are fully written by the casts below)
            for dw in DWS:
                nc.vector.memset(xsh_all[:, dw + 2, :, 0:2, :], 0.0)
                nc.vector.memset(xsh_all[:, dw + 2, :, 2 + H:HP, :], 0.0)
            nc.vector.memset(xsh_all[:, 3, :, 2:2 + H, W - 1:W], 0.0)
            nc.vector.memset(xsh_all[:, 4, :, 2:2 + H, W - 2:W], 0.0)
            nc.vector.memset(xsh_all[:, 1, :, 2:2 + H, 0:1], 0.0)
            nc.vector.memset(xsh_all[:, 0, :, 2:2 + H, 0:2], 0.0)

            # ---- input + weight DMA ----
            x_r = x_d[:].rearrange("one c h w -> (one c) h w")
            nc.sync.dma_start(out=xs_pad[:, 0, 2:2 + H, 2:2 + W],
                              in_=x_r[0:128, :, :])
            nc.scalar.dma_start(out=xs_pad[:, 1, 2:2 + H, 2:2 + W],
                                in_=x_r[128:256, :, :])
            nc.scalar.dma_start(out=wTf[:, :, :, :],
                                in_=wTt_d[:].rearrange("cc p k o -> p cc k o"))
            nc.scalar.dma_start(out=woffTf[:, :, :, :],
                                in_=woffTt_d[:].rearrange("cc p k o -> p cc k o"))
            nc.scalar.dma_start(out=fc1T[:, :, :],
                                in_=fc1t_d[:].rearrange("cc p m -> p cc m"))
            nc.vector.memset(fc2T[:, :], 0.0)
            nc.scalar.dma_start(out=fc2T[0:RED, :], in_=fc2t_d[:])
            nc.sync.dma_start(out=bconv[:, :],
                              in_=bconv_d[:].rearrange("(a c) -> c a", a=2))
            nc.sync.dma_start(out=boff[:, 0:1],
                              in_=boffd_d[:].rearrange("(c a) -> c a", a=1))
            nc.scalar.activation(boffn[:, 0:1], boff[:, 0:1], AF.Copy, scale=-1.0)

            # weight casts to bf16 (DVE)
            nc.vector.tensor_copy(wT[:, :, :, :], wTf[:, :, :, :])
            nc.vector.tensor_copy(woffT[:, :, :, :], woffTf[:, :, :, :])

            # ---- SE gate: s = sigmoid(fc2 @ relu(fc1 @ mean(x))) ----
            for cc in range(2):
                nc.vector.tensor_reduce(out=y_se[:, cc, 0:1],
                                        in_=xs_pad[:, cc, 2:2 + H, 2:2 + W],
                                        axis=mybir.AxisListType.XY, op=ALU.add)
            nc.vector.tensor_scalar_mul(y_se[:, :, 0:1], y_se[:, :, 0:1], 1.0 / HW)
            h_ps = psum_pre.tile([128, RED], F32, tag="se")
            for cc in range(2):
                nc.tensor.matmul(h_ps[0:RED, 0:1], lhsT=fc1T[:, cc, :],
                                 rhs=y_se[:, cc, 0:1],
                                 start=(cc == 0), stop=(cc == 1))
            nc.vector.memset(h_se[:, :], 0.0)
            nc.vector.tensor_relu(h_se[0:RED, 0:1], h_ps[0:RED, 0:1])
            for cc in range(2):
                s_ps = psum_pre.tile([128, RED], F32, tag="se")
                nc.tensor.matmul(s_ps[:, 0:1], lhsT=fc2T[:, cc * 128:(cc + 1) * 128],
                                 rhs=h_se[:, 0:1], start=True, stop=True)
                nc.scalar.activation(s_se[:, cc, 0:1], s_ps[:, 0:1], AF.Sigmoid)

            # fold the SE gate into the conv weights (contraction-side scale)
            for cc in range(2):
                nc.vector.tensor_scalar_mul(wT[:, cc, :, :], wT[:, cc, :, :],
                                            s_se[:, cc, 0:1])
                nc.vector.tensor_scalar_mul(woffT[:, cc, :, :], woffT[:, cc, :, :],
                                            s_se[:, cc, 0:1])

            # cast x to bf16, writing each column-shifted tile directly
            # (offset-conv shifts -1/0/1 first; they gate the PE)
            for dw in (-1, 0, 1, -2, 2):
                lo, hi = max(0, -dw), W - max(0, dw)
                for cc in range(2):
                    nc.vector.tensor_copy(
                        xsh_all[:, dw + 2, cc, 2:2 + H, lo:hi],
                        xs_pad[:, cc, 2:2 + H, 2 + dw + lo:2 + dw + hi])

            # ---- offset conv (standard 3x3, pad 1): all chunks ----
            for q in range(4):
                for nn in range(2):
                    off_ps = psum_pre.tile([64, NN], F32, tag="off")
                    for kk in range(K2):
                        ki, kj = divmod(kk, 3)
                        dh, dw = ki - 1, kj - 1
                        for cc in range(2):
                            r0 = 2 + dh + q * QROWS + nn * (QROWS // 2)
                            rhs = xsh_all[:, dw + 2, cc, r0:r0 + QROWS // 2, :]
                            nc.tensor.matmul(off_ps[0:64, :],
                                             lhsT=woffT[:, cc, kk, 0:64], rhs=rhs,
                                             start=(kk == 0 and cc == 0),
                                             stop=(kk == K2 - 1 and cc == 1))
                    ct = chunk_tiles()
                    wyx[(q, nn)] = ct
                    nc.scalar.activation(ct["wy0"][:, :], off_ps[0:K2, :], AF.Relu,
                                         scale=-1.0, bias=boffn[0:K2, 0:1])
                    nc.scalar.activation(ct["wy2"][:, :], off_ps[0:K2, :], AF.Relu,
                                         scale=1.0, bias=boff[0:K2, 0:1])
                    nc.scalar.activation(ct["wx0"][:, :], off_ps[32:32 + K2, :], AF.Relu,
                                         scale=-1.0, bias=boffn[32:32 + K2, 0:1])
                    nc.scalar.activation(ct["wx2"][:, :], off_ps[32:32 + K2, :], AF.Relu,
                                         scale=1.0, bias=boff[32:32 + K2, 0:1])
                    # map products go out immediately for every chunk:
                    # phase-1 DVE is idle while the PE runs the offset conv
                    emit_map_chunk(q, nn)

        # ================= main deform-conv loop =================
        with tc.tile_pool(name="mrep", bufs=6) as mreppool, \
             tc.tile_pool(name="zrep", bufs=3) as zreppool, \
             tc.tile_pool(name="mpool", bufs=3) as mpool, \
             tc.tile_pool(name="zmpool", bufs=4) as zmpool, \
             tc.tile_pool(name="zpool", bufs=2) as zpool, \
             tc.tile_pool(name="outpool", bufs=2) as outpool, \
             tc.tile_pool(name="psum_main", bufs=2, space="PSUM") as psum_main:

            def bcast_rows(pool, q, row0, rstride, cnt=3):
                """Broadcast `cnt` tap-map rows [QN] each to 128 partitions."""
                mrep3 = pool.tile([128, 3, QN], BF16)
                md = maps_dram[q][0:1, 0:1]  # anchor for tensor/offset
                next(rr).dma_start(
                    out=mrep3[:, 0:cnt, :],
                    in_=bass.AP(tensor=md.tensor, offset=md.offset + row0 * QN,
                                ap=[[0, 128], [rstride * QN, cnt], [1, QN]]))
                return mrep3

            PLANE = 2 * HP * W
            HPW = HP * W
            xsa = xsh_all[:, :, :, :, :]

            def mod_mulg(out_ap, mrep3, dw0, r0, step, cnt):
                """out[:, i, cc, n] = x_window(i) (*) map(i): cnt taps at once.

                The windows step regularly through the merged shifted-x tile
                (`step` elements per tap).  One DVE op covers the group.
                """
                xs_win = bass.AP(
                    tensor=xsa.tensor,
                    offset=xsa.offset + (dw0 + 2) * PLANE + r0 * W,
                    ap=[list(xsa.ap[0]), [step, cnt], [HPW, 2], [1, QN]])
                mrep_b = _bcast_ap(mrep3[:, 0, :], [[QN, cnt], [0, 2], [1, QN]])
                nc.vector.tensor_tensor(out_ap, xs_win, mrep_b, op=ALU.mult)

            def z_chain_steps(q):
                """Generator yielding one z tap-row emission at a time.

                The z positions share one [128, NZ, 2, QN] tile; each tap
                row is ONE grouped DVE multiply plus accumulate-DMAs.
                """
                zt3 = zpool.tile([128, NZ, 2, QN], BF16, tag="zt3", name="zt3")
                z_tiles[q] = zt3
                for ti, (r, s) in enumerate(TAPS):
                    mrep3 = bcast_rows(zreppool, q, ti * K2 + Z_KS[0],
                                       Z_KS[1] - Z_KS[0], cnt=NZ)
                    r0 = r + q * QROWS
                    if ti == 0:
                        mod_mulg(zt3[:, :, :, :], mrep3, s - 2, r0,
                                 2 * (PLANE + W), NZ)
                    else:
                        zm3 = zmpool.tile([128, NZ, 2, QN], BF16)
                        mod_mulg(zm3[:, :, :, :], mrep3, s - 2, r0,
                                 2 * (PLANE + W), NZ)
                        for i in range(NZ):
                            nc.gpsimd.dma_start(out=zt3[:, i, :, :],
                                                in_=zm3[:, i, :, :],
                                                accum_op=ALU.add)
                    yield

            z_tiles = {}
            # z(0) is built during the offset-conv window, before the main loop
            for _ in z_chain_steps(0):
                pass

            out_r = out_d[:].rearrange("one o h w -> (one o) h w")
            for q in range(4):
                ev_seq = EV_Q0 if q == 0 else EV_STD
                # background emissions interleaved into this quarter's stream
                bg = []
                if q < 3:
                    bg.append(z_chain_steps(q + 1))

                def emit_bg(n):
                    for _ in range(n):
                        while bg:
                            try:
                                next(bg[0])
                                break
                            except StopIteration:
                                bg.pop(0)
                        if not bg:
                            return

                ps = [psum_main.tile([128, QN], F32, tag=f"ps{oc}", name=f"ps{oc}")
                      for oc in range(2)]

                def do_mms(rhs_fn, kk, ev, tap, ntaps):
                    first = (ev == 0 and tap == 0)
                    last = (ev == len(ev_seq) - 1 and tap == ntaps - 1)
                    for cc in range(2):
                        for oc in range(2):
                            for n0, n1 in ((0, 512), (512, QN)):
                                nc.tensor.matmul(
                                    ps[oc][:, n0:n1],
                                    lhsT=wT[:, cc, kk, oc * 128:(oc + 1) * 128],
                                    rhs=rhs_fn(cc, n0, n1),
                                    start=(first and cc == 0),
                                    stop=(last and cc == 1))

                for ev, (kind, kk) in enumerate(ev_seq):
                    if kind == "z":
                        zt3 = z_tiles[q]
                        zi = Z_IDX[kk]
                        do_mms(lambda cc, n0, n1, _z=zt3, _i=zi:
                               _z[:, _i, cc, n0:n1], kk, ev, 0, 1)
                    else:
                        ki, kj = divmod(kk, 3)
                        for j in range(3):
                            mrep3 = bcast_rows(mreppool, q, 27 * j + kk, K2)
                            m3 = mpool.tile([128, 3, 2, QN], BF16)
                            mod_mulg(m3[:, :, :, :], mrep3, kj - 2,
                                     ki + j + q * QROWS, PLANE, 3)
                            for i in range(3):
                                tap = 3 * j + i
                                do_mms(lambda cc, n0, n1, _m=m3, _i=i:
                                       _m[:, _i, cc, n0:n1], kk, ev, tap, K2)
                                if tap % 4 == 3:
                                    emit_bg(1)

                for oc in range(2):
                    osb = outpool.tile([128, QN], F32)
                    nc.scalar.activation(osb[:, :], ps[oc][:, :], AF.Tanh,
                                         bias=bconv[:, oc:oc + 1])
                    nc.sync.dma_start(
                        out=out_r[oc * 128:(oc + 1) * 128,
                                  q * QROWS:(q + 1) * QROWS, :],
                        in_=osb[:, :])
                emit_bg(100)  # drain any leftover background work
    nc.finalize()
    return nc


_NC = None


def _get_nc():
    global _NC
    if _NC is None:
        _NC = build()
    return _NC


def _prep_host(inputs):
    """Host-side weight transposes (input staging)."""
    w_conv = np.asarray(inputs["w_conv"], dtype=np.float32)
    w_off = np.asarray(inputs["w_off"], dtype=np.float32)
    b_off = np.asarray(inputs["b_off"], dtype=np.float32)
    fc1 = np.asarray(inputs["fc1"], dtype=np.float32)
    fc2 = np.asarray(inputs["fc2"], dtype=np.float32)

    wc = w_conv.reshape(O, C, K2).transpose(1, 2, 0)        # [c, k, o]
    wTt = np.ascontiguousarray(wc.reshape(2, 128, K2, O))

    wo = w_off.reshape(2 * K2, C, K2)
    woffTt = np.zeros((2, 128, K2, 64), dtype=np.float32)
    dy = wo[0::2].transpose(1, 2, 0).reshape(2, 128, K2, K2)  # [cc,p,k,j]
    dx = wo[1::2].transpose(1, 2, 0).reshape(2, 128, K2, K2)
    woffTt[:, :, :, 0:K2] = dy
    woffTt[:, :, :, 32:32 + K2] = dx

    boffd = np.zeros(64, dtype=np.float32)
    boffd[0:K2] = b_off[0::2]
    boffd[32:32 + K2] = b_off[1::2]

    fc1t = np.ascontiguousarray(fc1.T.reshape(2, 128, RED))
    fc2t = np.ascontiguousarray(fc2.T)                       # [RED, C]
    return {"wTt": wTt, "woffTt": np.ascontiguousarray(woffTt),
            "boffd": boffd, "fc1t": fc1t, "fc2t": fc2t,
            "b_conv": np.ascontiguousarray(np.asarray(inputs["b_conv"],
                                                      dtype=np.float32))}


def kernel(**inputs):
    global LAST_RESULT
    from concourse.bass_utils import run_bass_kernel_spmd

    nc = _get_nc()
    x = np.ascontiguousarray(inputs["x"], dtype=np.float32)
    shared = _prep_host(inputs)
    in_maps = [{"x": x[i:i + 1], **shared} for i in range(B)]
    res = run_bass_kernel_spmd(nc, in_maps, core_ids=list(range(B)),
                               trace=bool(int(os.environ.get("KB_TRACE", "0"))))
    LAST_RESULT = res
    out = np.concatenate([res.results[i]["out"] for i in range(B)], axis=0)
    return out.astype(np.float32)


if __name__ == "__main__":
    nc = build()
    print("build OK")
